# revision 1
# baseline (speedup 1.0000x reference)
"""Trainium2 Bass kernel for nn_DinoText (retrieval_knn).

Computation (reference):
    t = l2norm(tanh(textual @ W.T + b))              [B, Dd]
    v = l2norm(visual, axis=-1)                      [B, P, Dd]
    sims = einsum('ik,ijk->ij', t, v); softmax; argmax -> idx  [B]
    v_best = v[b, idx[b]]                            [B, Dd]
    out = t @ v_best.T                               [B, B]

Strategy: data-parallel over batch across 8 NeuronCores (128 images each).
Per image, stream the [256, 1024] patch block through SBUF once:
  - ScalarE (ACT):  Square + accum_out  -> per-patch squared norms
  - VectorE (DVE):  tensor_tensor_reduce with a PE-broadcast t row -> dots
softmax is monotonic so argmax(softmax(s)) == argmax(s); the cosine score
s/sqrt(n) is compared via the monotone transform u = s*|s|/n (division-free
sign-preserving square) so no sqrt is needed in the argmax.
Winning rows are re-fetched with an indirect (gather) DMA, normalized, and
AllGathered (2 groups of 64 images, overlapping with the main stream).  The
final [128, 1024] x [1024, 1024]^T matmul runs on TensorE.
"""

import numpy as np

try:
    import concourse.bass as bass
except ImportError:  # toolchain lives in /opt in this container
    import sys

    for _p in ("/opt/pypackages", "/opt/trn_rl_repo"):
        if _p not in sys.path:
            sys.path.insert(0, _p)
    import concourse.bass as bass

import concourse.bacc as bacc
import concourse.mybir as mybir
import concourse.tile as tile
from concourse.bass_utils import run_bass_kernel_spmd
from concourse.masks import make_identity

NCORES = 8
B, P, DD, DC = 1024, 256, 1024, 512
BS = B // NCORES  # images per core
G = 32            # images per AllGather group
NG = BS // G      # groups per core
IPT = 1           # images per DMA tile

AF = mybir.ActivationFunctionType
ALU = mybir.AluOpType
F32 = mybir.dt.float32
I32 = mybir.dt.int32


def _build_kernel(tc, v_d, x_d, w_d, b_d, o_d):
    nc = tc.nc
    from contextlib import ExitStack

    ctx = ExitStack()
    # allocated first: its SBUF zone must not overlap the prep pools, or the
    # first v prefetches would wait for the prep-zone release
    vpool = ctx.enter_context(tc.tile_pool(name="vload", bufs=10))
    const = ctx.enter_context(tc.tile_pool(name="const", bufs=1))
    persist = ctx.enter_context(tc.tile_pool(name="persist", bufs=1))
    psum_bc = ctx.enter_context(tc.tile_pool(name="psbc", bufs=2, space="PSUM"))
    psum_tp = ctx.enter_context(tc.tile_pool(name="pstp", bufs=3, space="PSUM"))
    psum_s = ctx.enter_context(tc.tile_pool(name="pss", bufs=1, space="PSUM"))
    dram = ctx.enter_context(tc.tile_pool(name="dram", bufs=1, space="DRAM"))

    # ---- constants -------------------------------------------------------
    ident = const.tile([128, 128], F32, tag="ident")
    make_identity(nc, ident[:])
    ones_col = const.tile([1, 128], F32, tag="ones_col")
    nc.vector.memset(ones_col[:], 1.0)

    rowbase = []
    for g in range(NG):
        # rowbase[g][0, i] = (G*g + i) * 256: patch-row base of image G*g+i
        rbi = const.tile([1, G], I32, tag=f"rbi{g}", name=f"rbi{g}")
        nc.gpsimd.iota(
            rbi[:], pattern=[[256, G]], base=G * g * 256, channel_multiplier=0
        )
        rb = const.tile([1, G], F32, tag=f"rb{g}", name=f"rb{g}")
        nc.vector.tensor_copy(rb[:], rbi[:])
        rowbase.append(rb)

    # ---- phase 0: t_norm = l2norm(tanh(x @ W.T + b)) ---------------------
    t_norm = persist.tile([128, DD], F32, tag="t_norm")
    tT = [persist.tile([128, 128], F32, tag=f"tT{k}", name=f"tT{k}") for k in range(8)]

    with tc.tile_pool(name="prep", bufs=2) as prep, tc.tile_pool(name="wtp", bufs=1) as wtp:
        wT = [wtp.tile([128, DD], F32, tag=f"wT{j}", name=f"wT{j}") for j in range(4)]
        xT = [wtp.tile([128, 128], F32, tag=f"xT{j}", name=f"xT{j}") for j in range(4)]
        for kc in range(8):
            wn = prep.tile([128, DC], F32, tag="wn")
            nc.sync.dma_start(out=wn[:], in_=w_d[kc * 128 : (kc + 1) * 128, :])
            for j in range(4):
                pt = psum_tp.tile([128, 128], F32, tag="tp")
                nc.tensor.transpose(
                    out=pt[:], in_=wn[:, j * 128 : (j + 1) * 128], identity=ident[:]
                )
                nc.vector.tensor_copy(wT[j][:, kc * 128 : (kc + 1) * 128], pt[:])

        xn = prep.tile([128, DC], F32, tag="xn")
        nc.sync.dma_start(out=xn[:], in_=x_d[:, :])
        for j in range(4):
            pt = psum_tp.tile([128, 128], F32, tag="tp")
            nc.tensor.transpose(
                out=pt[:], in_=xn[:, j * 128 : (j + 1) * 128], identity=ident[:]
            )
            nc.vector.tensor_copy(xT[j][:], pt[:])

        bsb = const.tile([1, DD], F32, tag="bsb")
        nc.sync.dma_start(out=bsb[:], in_=b_d[:, :])

        t_sb = prep.tile([128, DD], F32, tag="t_sb")
        for h in range(2):
            tp_ps = psum_s.tile([128, 512], F32, tag="tps")
            for j in range(4):
                nc.tensor.matmul(
                    out=tp_ps[:],
                    lhsT=xT[j][:],
                    rhs=wT[j][:, h * 512 : (h + 1) * 512],
                    start=(j == 0),
                    stop=False,
                )
            nc.tensor.matmul(
                out=tp_ps[:],
                lhsT=ones_col[:],
                rhs=bsb[:, h * 512 : (h + 1) * 512],
                start=False,
                stop=True,
            )
            nc.scalar.activation(
                out=t_sb[:, h * 512 : (h + 1) * 512], in_=tp_ps[:], func=AF.Tanh
            )

        tn2 = const.tile([128, 1], F32, tag="tn2")
        tscr = prep.tile([128, DD], F32, tag="tscr")
        nc.vector.scalar_tensor_tensor(
            out=tscr[:],
            in0=t_sb[:],
            scalar=0.0,
            in1=t_sb[:],
            op0=ALU.bypass,
            op1=ALU.mult,
            accum_out=tn2[:],
        )
        tinv = const.tile([128, 1], F32, tag="tinv")
        nc.vector.reciprocal(tinv[:], tn2[:])
        trsq = const.tile([128, 1], F32, tag="trsq")
        nc.scalar.activation(out=trsq[:], in_=tinv[:], func=AF.Sqrt)
        nc.scalar.activation(out=t_norm[:], in_=t_sb[:], func=AF.Copy, scale=trsq[:])

        for kc in range(8):
            pt = psum_tp.tile([128, 128], F32, tag="tp")
            nc.tensor.transpose(
                out=pt[:], in_=t_norm[:, kc * 128 : (kc + 1) * 128], identity=ident[:]
            )
            nc.vector.tensor_copy(tT[kc][:], pt[:])

    # ---- group accumulators / AllGather bounces --------------------------
    sims_g = [persist.tile([128, 2 * G], F32, tag=f"sims{g}", name=f"sims{g}") for g in range(NG)]
    norms_g = [persist.tile([128, 2 * G], F32, tag=f"norms{g}", name=f"norms{g}") for g in range(NG)]
    ag_in = [dram.tile([G, DD], F32, tag=f"agin{g}", name=f"agin{g}") for g in range(NG)]
    t_dram = dram.tile([BS, DD], F32, tag="tdram", name="tdram")
    # gpsimd queue: keeps both the SP v-load stream and the ACT square
    # stream free of this serial dependency
    nc.gpsimd.dma_start(out=t_dram[:], in_=t_norm[:])
    import os as _os
    _ag_space = "Local" if _os.environ.get("DINO_NO_COLLECTIVE") == "1" else "Shared"
    ag_out = [
        dram.tile([G * NCORES, DD], F32, tag=f"agout{g}", name=f"agout{g}", addr_space=_ag_space)
        for g in range(NG)
    ]

    dscr = ctx.enter_context(tc.tile_pool(name="dscr", bufs=1))
    nscr = ctx.enter_context(tc.tile_pool(name="nscr", bufs=1))
    gp = ctx.enter_context(tc.tile_pool(name="gp", bufs=1))
    trowp = ctx.enter_context(tc.tile_pool(name="trow", bufs=2))

    v_ap = v_d  # [BS, P, DD]
    v_flat = v_d.rearrange("b p k -> (b p) k")

    vbaT = [persist.tile([128, 1024], F32, tag=f"vbaT{k}", name=f"vbaT{k}") for k in range(8)]
    s_sb = persist.tile([128, B], F32, tag="s_sb")
    # output view: S_sb column jp = (g, cc, ig) -> output column 128*cc + G*g + ig
    o_view = o_d.rearrange("p (cc gg ig) -> p gg cc ig", cc=NCORES, gg=NG)
    ldp2 = ctx.enter_context(tc.tile_pool(name="ld2", bufs=2))
    NSUB = 128 // G  # cc-blocks per 128-row chunk of ag_out
    NQ = (G * NCORES) // 128

    def gather_back(g):
        """load this group's AllGather result, transpose into vbaT, and run
        this group's share of the final S matmul.  DMAs ride the gpsimd
        queue; for early groups everything overlaps the main stream."""
        # vbaT columns live in gathered order jp = 256*g + 128*q + r
        # (image j = 128*cc + G*g + ig); the final output DMA permutes
        for q in range(NQ):
            ld = ldp2.tile([128, DD], F32, tag="ld")
            nc.gpsimd.dma_start(
                out=ld[:], in_=ag_out[g][q * 128 : (q + 1) * 128, :]
            )
            for kc in range(8):
                pt = psum_tp.tile([128, 128], F32, tag="tp")
                nc.tensor.transpose(
                    out=pt[:],
                    in_=ld[:, kc * 128 : (kc + 1) * 128],
                    identity=ident[:],
                )
                j0 = NCORES * G * g + 128 * q
                if (kc + q) % 2 == 0:
                    nc.vector.tensor_copy(vbaT[kc][:, j0 : j0 + 128], pt[:])
                else:
                    nc.scalar.copy(vbaT[kc][:, j0 : j0 + 128], pt[:])
        # partial S for this group's contiguous column block
        CG = NCORES * G
        spg = psum_s.tile([128, CG], F32, tag="tps")
        for kc in range(8):
            nc.tensor.matmul(
                out=spg[:],
                lhsT=tT[kc][:],
                rhs=vbaT[kc][:, CG * g : CG * (g + 1)],
                start=(kc == 0),
                stop=(kc == 7),
            )
        nc.scalar.activation(
            out=s_sb[:, CG * g : CG * (g + 1)], in_=spg[:], func=AF.Copy
        )
        nc.gpsimd.dma_start(
            out=o_view[:, g], in_=s_sb[:, CG * g : CG * (g + 1)]
        )

    def process_group(g):
        """argmax over the group's 256 scores per image, gather + normalize
        winners, kick off the AllGather."""
        rn = gp.tile([128, 2 * G], F32, tag="rn")
        nc.vector.reciprocal(rn[:], norms_g[g][:])
        sneg = gp.tile([128, 2 * G], F32, tag="sneg")
        nc.vector.tensor_scalar_mul(sneg[:], sims_g[g][:], -1.0)
        sabs = gp.tile([128, 2 * G], F32, tag="sabs")
        nc.vector.tensor_tensor(sabs[:], sims_g[g][:], sneg[:], op=ALU.max)
        rat = gp.tile([128, 2 * G], F32, tag="rat")
        nc.vector.tensor_tensor(rat[:], sims_g[g][:], rn[:], op=ALU.mult)
        u = gp.tile([128, 2 * G], F32, tag="u")
        nc.vector.tensor_tensor(u[:], rat[:], sabs[:], op=ALU.mult)

        R2 = 2 * G  # rows of the transposed score tile: (parity, ig)
        ptu = psum_tp.tile([128, 128], F32, tag="tp")
        nc.tensor.transpose(out=ptu[0:R2, :], in_=u[:], identity=ident[:])
        uT = gp.tile([R2, 128], F32, tag="uT")
        nc.vector.tensor_copy(uT[:], ptu[0:R2, :])

        mx = gp.tile([R2, 8], F32, tag="mx")
        mi = gp.tile([R2, 8], mybir.dt.uint32, tag="mi")
        nc.vector.max_with_indices(out_max=mx[:], out_indices=mi[:], in_=uT[:])
        mif = gp.tile([R2, 1], F32, tag="mif")
        nc.vector.tensor_copy(mif[:], mi[:, 0:1])

        # bring the per-(parity, image) maxes/indices onto partition 0 via
        # PE transposes (engines can't mix SBUF base partitions)
        R2 = 2 * G
        ptm = psum_tp.tile([1, R2], F32, tag="tp")
        nc.tensor.transpose(out=ptm[:], in_=mx[:, 0:1], identity=ident[0:R2, 0:R2])
        mxT = gp.tile([1, R2], F32, tag="mxT")
        nc.vector.tensor_copy(mxT[:], ptm[:])
        pti = psum_tp.tile([1, R2], F32, tag="tp")
        nc.tensor.transpose(out=pti[:], in_=mif[:], identity=ident[0:R2, 0:R2])
        miT = gp.tile([1, R2], F32, tag="miT")
        nc.vector.tensor_copy(miT[:], pti[:])

        m0, m1 = mxT[0:1, 0:G], mxT[0:1, G : 2 * G]
        i0, i1 = miT[0:1, 0:G], miT[0:1, G : 2 * G]
        gt = gp.tile([1, G], mybir.dt.uint32, tag="gt")
        nc.vector.tensor_tensor(gt[:], m1, m0, op=ALU.is_gt)
        gtf = gp.tile([1, G], F32, tag="gtf")
        nc.vector.tensor_copy(gtf[:], gt[:])
        isel = gp.tile([1, G], F32, tag="isel")
        nc.vector.tensor_copy(isel[:], i0)
        nc.vector.copy_predicated(isel[:], gt[:], i1)
        # patch index = 2*isel + parity; add per-image patch-row base
        grow = gp.tile([1, G], F32, tag="grow")
        nc.vector.scalar_tensor_tensor(
            out=grow[:], in0=isel[:], scalar=2.0, in1=gtf[:],
            op0=ALU.mult, op1=ALU.add,
        )
        nc.vector.tensor_tensor(grow[:], grow[:], rowbase[g][:], op=ALU.add)
        ptg = psum_tp.tile([G, 1], F32, tag="tp")
        nc.tensor.transpose(out=ptg[:], in_=grow[:], identity=ident[0:1, 0:1])
        gidxf = gp.tile([G, 1], F32, tag="gidxf")
        nc.vector.tensor_copy(gidxf[:], ptg[:])
        gidx = gp.tile([G, 1], I32, tag="gidx")
        nc.vector.tensor_copy(gidx[:], gidxf[:])

        vb = gp.tile([G, DD], F32, tag="vb")
        import os as _os

        if _os.environ.get("DINO_NO_GATHER") == "1":
            # debug: fixed gather (patch 0 of each image) — wrong result,
            # exercises everything but the indirect DMA
            nc.sync.dma_start(
                out=vb[:],
                in_=v_ap[G * g : G * (g + 1)].rearrange("b p k -> b p k")[:, 0, :],
            )
        else:
            nc.gpsimd.indirect_dma_start(
                out=vb[:],
                out_offset=None,
                in_=v_flat,
                in_offset=bass.IndirectOffsetOnAxis(ap=gidx[:], axis=0),
            )
        vbs = gp.tile([G, DD], F32, tag="vbs")
        nb2 = gp.tile([G, 1], F32, tag="nb2")
        nc.scalar.activation(out=vbs[:], in_=vb[:], func=AF.Square, accum_out=nb2[:])
        nbr = gp.tile([G, 1], F32, tag="nbr")
        nc.vector.reciprocal(nbr[:], nb2[:])
        nbs = gp.tile([G, 1], F32, tag="nbs")
        nc.scalar.activation(out=nbs[:], in_=nbr[:], func=AF.Sqrt)
        vbn = gp.tile([G, DD], F32, tag="vbn")
        nc.scalar.activation(out=vbn[:], in_=vb[:], func=AF.Copy, scale=nbs[:])
        # issue on the gpsimd (SWDGE) queue: SP/ACT sequencers carry the
        # main stream and must not block on this chain
        nc.gpsimd.dma_start(out=ag_in[g][:], in_=vbn[:])
        import os as _os

        if _os.environ.get("DINO_NO_COLLECTIVE") == "1":
            # debug: skip the collective; replicate local shard into all slots
            for cc in range(NCORES):
                nc.gpsimd.dma_start(
                    out=ag_out[g][cc * G : (cc + 1) * G, :], in_=ag_in[g][:]
                )
        else:
            nc.gpsimd.collective_compute(
                "AllGather",
                ALU.bypass,
                replica_groups=[list(range(NCORES))],
                ins=[ag_in[g][:].opt()],
                outs=[ag_out[g][:].opt()],
            )

    # ---- main stream -----------------------------------------------------
    _phase = _os.environ.get("DINO_PHASE", "full")
    if _phase == "prep":
        # debug: write t_norm and stop
        nc.sync.dma_start(out=o_d[:, :], in_=t_norm[:])
        ctx.close()
        return
    RQ = 4  # t rows staged per DMA chunk
    trow8 = None
    for ii in range(0, BS, IPT):
        if ii % RQ == 0:
            # engine operands must start at partition 0/32/64/96, so stage
            # t_norm rows at partition 0 in chunks via a DRAM round trip
            # (one DMA per RQ images; per-image SBUF->SBUF row DMAs would
            # saturate the issuing sequencer)
            trow8 = trowp.tile([1, RQ, DD], F32, tag="trow")
            nc.gpsimd.dma_start(
                out=trow8[:], in_=t_dram[ii : ii + RQ, :]
            )
        # partition p holds patches (2p, 2p+1) of each image: 8KB contiguous
        # HBM runs per (partition, image) -> half the DMA descriptors
        vt = vpool.tile([128, IPT, 2 * DD], F32, tag="vt")
        src = v_ap[ii : ii + IPT].rearrange("b (p j) k -> p b (j k)", p=128)
        nc.sync.dma_start(out=vt[:], in_=src)
        for bb in range(IPT):
            i = ii + bb
            g, ig = i // G, i % G
            bc = psum_bc.tile([128, DD], F32, tag="bc")
            for h in range(2):
                nc.tensor.matmul(
                    out=bc[:, h * 512 : (h + 1) * 512],
                    lhsT=ones_col[:],
                    rhs=trow8[0:1, i % RQ, h * 512 : (h + 1) * 512],
                    start=True,
                    stop=True,
                )
            sd = dscr.tile([128, DD], F32, tag="sd")
            sn = nscr.tile([128, DD], F32, tag="sn")
            for c2 in range(2):  # patch parity: partition p <-> patch 2p+c2
                col = c2 * G + ig
                nc.vector.scalar_tensor_tensor(
                    out=sd[:],
                    in0=vt[:, bb, c2 * DD : (c2 + 1) * DD],
                    scalar=0.0,
                    in1=bc[:],
                    op0=ALU.bypass,
                    op1=ALU.mult,
                    accum_out=sims_g[g][:, col : col + 1],
                )
                nc.scalar.activation(
                    out=sn[:],
                    in_=vt[:, bb, c2 * DD : (c2 + 1) * DD],
                    func=AF.Square,
                    accum_out=norms_g[g][:, col : col + 1],
                )
        done = ii + IPT
        if _phase != "main" and done >= G + 8 and (done - 8) % G == 0 and done < BS:
            process_group((done - 8) // G - 1)
        if _phase != "main" and done >= G + 24 and (done - 24) % G == 0 and done < BS:
            gather_back((done - 24) // G - 1)
    if _phase == "main":
        # debug: write the raw accumulators and stop
        nc.sync.dma_start(out=o_d[:, 0:128], in_=sims_g[0][:])
        nc.sync.dma_start(out=o_d[:, 128:256], in_=norms_g[0][:])
        nc.sync.dma_start(out=o_d[:, 256:384], in_=sims_g[1][:])
        nc.sync.dma_start(out=o_d[:, 384:512], in_=norms_g[1][:])
        ctx.close()
        return
    process_group(NG - 1)
    for _g in range(NG):
        if _g * G + 24 >= BS or _g == NG - 1:
            gather_back(_g)

    ctx.close()


_CACHE = {}


def build():
    if "nc" in _CACHE:
        return _CACHE["nc"]
    nc = bacc.Bacc(
        "TRN2", target_bir_lowering=False, debug=False, num_devices=NCORES
    )
    v_d = nc.dram_tensor("v", [BS, P, DD], F32, kind="ExternalInput").ap()
    x_d = nc.dram_tensor("x", [BS, DC], F32, kind="ExternalInput").ap()
    w_d = nc.dram_tensor("w", [DD, DC], F32, kind="ExternalInput").ap()
    b_d = nc.dram_tensor("bv", [1, DD], F32, kind="ExternalInput").ap()
    o_d = nc.dram_tensor("out", [BS, DD], F32, kind="ExternalOutput").ap()
    with tile.TileContext(nc) as tc:
        _build_kernel(tc, v_d, x_d, w_d, b_d, o_d)
    nc.compile()
    _CACHE["nc"] = nc
    return nc


def make_in_maps(visual_embedding, textual_embedding, W, b):
    in_maps = []
    for c in range(NCORES):
        sl = slice(c * BS, (c + 1) * BS)
        in_maps.append(
            {
                "v": np.ascontiguousarray(visual_embedding[sl], dtype=np.float32),
                "x": np.ascontiguousarray(textual_embedding[sl], dtype=np.float32),
                "w": np.ascontiguousarray(W, dtype=np.float32),
                "bv": np.ascontiguousarray(b, dtype=np.float32).reshape(1, DD),
            }
        )
    return in_maps


def kernel(visual_embedding, textual_embedding, W, b, _trace=False):
    nc = build()
    in_maps = make_in_maps(visual_embedding, textual_embedding, W, b)
    res = run_bass_kernel_spmd(nc, in_maps, list(range(NCORES)), trace=_trace)
    out = np.concatenate([res.results[c]["out"] for c in range(NCORES)], axis=0)
    if _trace:
        kernel.last_exec_time_ns = res.exec_time_ns
        kernel.last_profile = res.profile_json
    return out



# revision 5
# speedup vs baseline: 1.2962x; 1.2962x over previous
"""Trainium2 Bass kernel for nn_DinoText (retrieval_knn).

Computation (reference):
    t = l2norm(tanh(textual @ W.T + b))              [B, Dd]
    v = l2norm(visual, axis=-1)                      [B, P, Dd]
    sims = einsum('ik,ijk->ij', t, v); softmax; argmax -> idx  [B]
    v_best = v[b, idx[b]]                            [B, Dd]
    out = t @ v_best.T                               [B, B]

Strategy: data-parallel over batch across 8 NeuronCores (BS=128 images
each).  SBUF partition = image (BS == 128), so the per-image text row
t_norm[i] sits on partition i and the patch stream needs NO broadcast:
  - VectorE:  (v_tile * t_norm) with accum_out -> per-patch dots
  - ScalarE:  Square with accum_out            -> per-patch sq-norms
softmax is monotonic so argmax(softmax(s)) == argmax(s); the cosine
score s/sqrt(n) is compared via the monotone transform u = s*|s|/n
(division-free sign-preserving square) so no sqrt is needed.
Each core computes the output COLUMNS for its own images:
    out[:, mine] = t_all @ v_best_mine.T
so the collective is an AllGather of the (transposed) text embeddings,
issued right after the prep phase and fully hidden under the ~375us
visual-embedding stream.  The tail (argmax, indirect gather of the 128
winning patch rows, normalize, 8 transposes + 64 matmuls, output DMA)
is ~25us of serial work.
"""

import numpy as np

try:
    import concourse.bass as bass
except ImportError:  # toolchain lives in /opt in this container
    import sys

    for _p in ("/opt/pypackages", "/opt/trn_rl_repo"):
        if _p not in sys.path:
            sys.path.insert(0, _p)
    import concourse.bass as bass

import concourse.bacc as bacc
import concourse.mybir as mybir
import concourse.tile as tile
from concourse.bass_utils import run_bass_kernel_spmd
from concourse.masks import make_identity

NCORES = 8
B, P, DD, DC = 1024, 256, 1024, 512
BS = B // NCORES  # images per core (= 128 = SBUF partitions)
IPP = 2           # patch-PAIRS per DMA tile (tile = [128, IPP*2, 1024] = IPP MB)
NT = P // (2 * IPP)  # stream iterations

AF = mybir.ActivationFunctionType
ALU = mybir.AluOpType
F32 = mybir.dt.float32
I32 = mybir.dt.int32

KB = 2 * IPP  # patches per tile


def _build_kernel(tc, v_d, x_d, w_d, b_d, o_d):
    nc = tc.nc
    from contextlib import ExitStack

    ctx = ExitStack()
    # vload first so its SBUF zone is independent of the prep pools
    vpool = ctx.enter_context(tc.tile_pool(name="vload", bufs=6))
    const = ctx.enter_context(tc.tile_pool(name="const", bufs=1))
    persist = ctx.enter_context(tc.tile_pool(name="persist", bufs=1))
    psum_tp = ctx.enter_context(tc.tile_pool(name="pstp", bufs=3, space="PSUM"))
    psum_s = ctx.enter_context(tc.tile_pool(name="pss", bufs=1, space="PSUM"))
    psum_o = ctx.enter_context(tc.tile_pool(name="pso", bufs=2, space="PSUM"))
    dram = ctx.enter_context(tc.tile_pool(name="dram", bufs=1, space="DRAM"))

    # ---- constants -------------------------------------------------------
    ident = const.tile([128, 128], F32, tag="ident")
    make_identity(nc, ident[:])
    ones_col = const.tile([1, 128], F32, tag="ones_col")
    nc.vector.memset(ones_col[:], 1.0)
    # rowbase[i, 0] = i * P  (flat patch-row base of image i), as f32 for
    # DVE index arithmetic (exact below 2^24)
    rowbase_i = const.tile([128, 1], I32, tag="rowbase_i")
    nc.gpsimd.iota(rowbase_i[:], pattern=[[0, 1]], base=0, channel_multiplier=P)
    rowbase = const.tile([128, 1], F32, tag="rowbase")
    nc.vector.tensor_copy(rowbase[:], rowbase_i[:])

    # ---- phase 0: t_norm = l2norm(tanh(x @ W.T + b)) ---------------------
    t_norm = persist.tile([128, DD], F32, tag="t_norm")
    tT = [persist.tile([128, 128], F32, tag=f"tT{k}", name=f"tT{k}") for k in range(8)]

    with tc.tile_pool(name="prep", bufs=2) as prep, tc.tile_pool(name="wtp", bufs=1) as wtp:
        wT = [wtp.tile([128, DD], F32, tag=f"wT{j}", name=f"wT{j}") for j in range(4)]
        xT = [wtp.tile([128, 128], F32, tag=f"xT{j}", name=f"xT{j}") for j in range(4)]
        for kc in range(8):
            wn = prep.tile([128, DC], F32, tag="wn")
            nc.sync.dma_start(out=wn[:], in_=w_d[kc * 128 : (kc + 1) * 128, :])
            for j in range(4):
                pt = psum_tp.tile([128, 128], F32, tag="tp")
                nc.tensor.transpose(
                    out=pt[:], in_=wn[:, j * 128 : (j + 1) * 128], identity=ident[:]
                )
                nc.vector.tensor_copy(wT[j][:, kc * 128 : (kc + 1) * 128], pt[:])

        xn = prep.tile([128, DC], F32, tag="xn")
        nc.sync.dma_start(out=xn[:], in_=x_d[:, :])
        for j in range(4):
            pt = psum_tp.tile([128, 128], F32, tag="tp")
            nc.tensor.transpose(
                out=pt[:], in_=xn[:, j * 128 : (j + 1) * 128], identity=ident[:]
            )
            nc.vector.tensor_copy(xT[j][:], pt[:])

        bsb = const.tile([1, DD], F32, tag="bsb")
        nc.sync.dma_start(out=bsb[:], in_=b_d[:, :])

        t_sb = prep.tile([128, DD], F32, tag="t_sb")
        for h in range(2):
            tp_ps = psum_s.tile([128, 512], F32, tag="tps")
            for j in range(4):
                nc.tensor.matmul(
                    out=tp_ps[:],
                    lhsT=xT[j][:],
                    rhs=wT[j][:, h * 512 : (h + 1) * 512],
                    start=(j == 0),
                    stop=False,
                )
            nc.tensor.matmul(
                out=tp_ps[:],
                lhsT=ones_col[:],
                rhs=bsb[:, h * 512 : (h + 1) * 512],
                start=False,
                stop=True,
            )
            nc.scalar.activation(
                out=t_sb[:, h * 512 : (h + 1) * 512], in_=tp_ps[:], func=AF.Tanh
            )

        tn2 = const.tile([128, 1], F32, tag="tn2")
        tscr = prep.tile([128, DD], F32, tag="tscr")
        nc.vector.scalar_tensor_tensor(
            out=tscr[:],
            in0=t_sb[:],
            scalar=0.0,
            in1=t_sb[:],
            op0=ALU.bypass,
            op1=ALU.mult,
            accum_out=tn2[:],
        )
        tinv = const.tile([128, 1], F32, tag="tinv")
        nc.vector.reciprocal(tinv[:], tn2[:])
        trsq = const.tile([128, 1], F32, tag="trsq")
        nc.scalar.activation(out=trsq[:], in_=tinv[:], func=AF.Sqrt)
        nc.scalar.activation(out=t_norm[:], in_=t_sb[:], func=AF.Copy, scale=trsq[:])

        for kc in range(8):
            pt = psum_tp.tile([128, 128], F32, tag="tp")
            nc.tensor.transpose(
                out=pt[:], in_=t_norm[:, kc * 128 : (kc + 1) * 128], identity=ident[:]
            )
            nc.vector.tensor_copy(tT[kc][:], pt[:])

    # ---- AllGather of tT: [1024, 128] per core -> [8192, 128] ------------
    ag_in = dram.tile([8 * 128, 128], F32, tag="agin", name="agin")
    for kc in range(8):
        nc.gpsimd.dma_start(out=ag_in[kc * 128 : (kc + 1) * 128, :], in_=tT[kc][:])
    import os as _os

    _ag_space = "Local" if _os.environ.get("DINO_NO_COLLECTIVE") == "1" else "Shared"
    ag_out = dram.tile(
        [NCORES * 8 * 128, 128], F32, tag="agout", name="agout", addr_space=_ag_space
    )
    if _os.environ.get("DINO_NO_COLLECTIVE") == "1":
        for cc in range(NCORES):
            nc.gpsimd.dma_start(
                out=ag_out[cc * 1024 : (cc + 1) * 1024, :], in_=ag_in[:]
            )
    else:
        nc.gpsimd.collective_compute(
            "AllGather",
            ALU.bypass,
            replica_groups=[list(range(NCORES))],
            ins=[ag_in[:].opt()],
            outs=[ag_out[:].opt()],
        )

    # gathered tT, loaded into SBUF during the stream: tile (r, kc) at
    # agT[:, 8*r + kc, :] = tT of rank r's images, k-chunk kc
    agT = persist.tile([128, 64, 128], F32, tag="agT")

    # ---- score / norm accumulators --------------------------------------
    sims = persist.tile([128, P], F32, tag="sims")
    norms = persist.tile([128, P], F32, tag="norms")

    dscr = ctx.enter_context(tc.tile_pool(name="dscr", bufs=1))
    nscr = ctx.enter_context(tc.tile_pool(name="nscr", bufs=1))
    gp = ctx.enter_context(tc.tile_pool(name="gp", bufs=1))

    v_flat = v_d.rearrange("b p k -> (b p) k")

    # ---- main stream: partitions = images, free = patches ----------------
    for it in range(NT):
        vt = vpool.tile([128, KB, DD], F32, tag="vt")
        nc.sync.dma_start(out=vt[:], in_=v_d[:, it * KB : (it + 1) * KB, :])
        for j in range(KB):
            p = it * KB + j
            sd = dscr.tile([128, DD], F32, tag="sd")
            nc.vector.scalar_tensor_tensor(
                out=sd[:],
                in0=vt[:, j, :],
                scalar=0.0,
                in1=t_norm[:],
                op0=ALU.bypass,
                op1=ALU.mult,
                accum_out=sims[:, p : p + 1],
            )
            sn = nscr.tile([128, DD], F32, tag="sn")
            nc.scalar.activation(
                out=sn[:],
                in_=vt[:, j, :],
                func=AF.Square,
                accum_out=norms[:, p : p + 1],
            )
        if it == NT // 2:
            # AllGather has completed by now; pull the gathered tT into
            # SBUF on the gpsimd queue (does not touch the sync DMA ring)
            nc.gpsimd.dma_start(
                out=agT[:], in_=ag_out[:].rearrange("(t q) c -> q t c", q=128)
            )

    # ---- tail: argmax, gather winners, normalize, final matmul -----------
    rn = gp.tile([128, P], F32, tag="rn")
    nc.vector.reciprocal(rn[:], norms[:])
    sneg = gp.tile([128, P], F32, tag="sneg")
    nc.vector.tensor_scalar_mul(sneg[:], sims[:], -1.0)
    sabs = gp.tile([128, P], F32, tag="sabs")
    nc.vector.tensor_tensor(sabs[:], sims[:], sneg[:], op=ALU.max)
    rat = gp.tile([128, P], F32, tag="rat")
    nc.vector.tensor_tensor(rat[:], sims[:], rn[:], op=ALU.mult)
    u = gp.tile([128, P], F32, tag="u")
    nc.vector.tensor_tensor(u[:], rat[:], sabs[:], op=ALU.mult)

    mx = gp.tile([128, 8], F32, tag="mx")
    mi = gp.tile([128, 8], mybir.dt.uint32, tag="mi")
    nc.vector.max_with_indices(out_max=mx[:], out_indices=mi[:], in_=u[:])
    mif = gp.tile([128, 1], F32, tag="mif")
    nc.vector.tensor_copy(mif[:], mi[:, 0:1])
    gf = gp.tile([128, 1], F32, tag="gf")
    nc.vector.tensor_tensor(gf[:], mif[:], rowbase[:], op=ALU.add)
    gidx = gp.tile([128, 1], I32, tag="gidx")
    nc.vector.tensor_copy(gidx[:], gf[:])

    vb = gp.tile([128, DD], F32, tag="vb")
    nc.gpsimd.indirect_dma_start(
        out=vb[:],
        out_offset=None,
        in_=v_flat,
        in_offset=bass.IndirectOffsetOnAxis(ap=gidx[:], axis=0),
    )
    vbs = gp.tile([128, DD], F32, tag="vbs")
    nb2 = gp.tile([128, 1], F32, tag="nb2")
    nc.scalar.activation(out=vbs[:], in_=vb[:], func=AF.Square, accum_out=nb2[:])
    nbr = gp.tile([128, 1], F32, tag="nbr")
    nc.vector.reciprocal(nbr[:], nb2[:])
    nbs = gp.tile([128, 1], F32, tag="nbs")
    nc.scalar.activation(out=nbs[:], in_=nbr[:], func=AF.Sqrt)
    vbn = gp.tile([128, DD], F32, tag="vbn")
    nc.scalar.activation(out=vbn[:], in_=vb[:], func=AF.Copy, scale=nbs[:])

    # vbT[kc] = [128 k, 128 my-images]
    vbT = [gp.tile([128, 128], F32, tag=f"vbT{k}", name=f"vbT{k}") for k in range(8)]
    for kc in range(8):
        pt = psum_tp.tile([128, 128], F32, tag="tp")
        nc.tensor.transpose(
            out=pt[:], in_=vbn[:, kc * 128 : (kc + 1) * 128], identity=ident[:]
        )
        nc.vector.tensor_copy(vbT[kc][:], pt[:])

    # out rows chunk r (rank r's images) = agT(r).T @ vbT
    o_sb = gp.tile([128, 8, 128], F32, tag="o_sb")
    for r in range(8):
        po = psum_o.tile([128, 128], F32, tag="po")
        for kc in range(8):
            nc.tensor.matmul(
                out=po[:],
                lhsT=agT[:, 8 * r + kc, :],
                rhs=vbT[kc][:],
                start=(kc == 0),
                stop=(kc == 7),
            )
        nc.scalar.activation(out=o_sb[:, r, :], in_=po[:], func=AF.Copy)
        nc.sync.dma_start(out=o_d[r * 128 : (r + 1) * 128, :], in_=o_sb[:, r, :])

    ctx.close()


_CACHE = {}


def build():
    if "nc" in _CACHE:
        return _CACHE["nc"]
    nc = bacc.Bacc(
        "TRN2", target_bir_lowering=False, debug=False, num_devices=NCORES
    )
    v_d = nc.dram_tensor("v", [BS, P, DD], F32, kind="ExternalInput").ap()
    x_d = nc.dram_tensor("x", [BS, DC], F32, kind="ExternalInput").ap()
    w_d = nc.dram_tensor("w", [DD, DC], F32, kind="ExternalInput").ap()
    b_d = nc.dram_tensor("bv", [1, DD], F32, kind="ExternalInput").ap()
    o_d = nc.dram_tensor("out", [B, BS], F32, kind="ExternalOutput").ap()
    with tile.TileContext(nc) as tc:
        _build_kernel(tc, v_d, x_d, w_d, b_d, o_d)
    nc.compile()
    _CACHE["nc"] = nc
    return nc


def make_in_maps(visual_embedding, textual_embedding, W, b):
    in_maps = []
    for c in range(NCORES):
        sl = slice(c * BS, (c + 1) * BS)
        in_maps.append(
            {
                "v": np.ascontiguousarray(visual_embedding[sl], dtype=np.float32),
                "x": np.ascontiguousarray(textual_embedding[sl], dtype=np.float32),
                "w": np.ascontiguousarray(W, dtype=np.float32),
                "bv": np.ascontiguousarray(b, dtype=np.float32).reshape(1, DD),
            }
        )
    return in_maps


def kernel(visual_embedding, textual_embedding, W, b, _trace=False):
    nc = build()
    in_maps = make_in_maps(visual_embedding, textual_embedding, W, b)
    res = run_bass_kernel_spmd(nc, in_maps, list(range(NCORES)), trace=_trace)
    out = np.concatenate([res.results[c]["out"] for c in range(NCORES)], axis=1)
    if _trace:
        kernel.last_exec_time_ns = res.exec_time_ns
        kernel.last_profile = res.profile_json
        kernel.last_trace = (
            res.instructions_and_trace[1] if res.instructions_and_trace else None
        )
    return out
